# revision 4
# baseline (speedup 1.0000x reference)
"""Trainium2 Bass kernel for nn_BilinearInterpolator (dense per-coord CNN).

Math (per (b, n) pair):
  u      = w1[:, :5] @ [image_b; pos]              # [64, 1024], shared over n
  v      = w1[:, 5:] @ coords[b, n] + b1           # [64] per-pair bias
  h1     = leaky(u + v)                            # [64, 1024]
  h_l    = leaky(W_l h_{l-1} + b_l)   l = 2..5
  pooled = mean_hw(h5);  out = sigmoid(wl @ pooled + bl)

Sharding: 512 (b, n) pairs data-parallel over 8 cores (64 pairs each; every
core owns a single b). On-chip layout packs 2 pairs per 128-partition tile
(channels 0-63 = even pair, 64-127 = odd pair); all matmuls use block-diagonal
[128, 128] weights.

v2 design:
- A custom DVE op (LEAKY_BIAS_ANT) computes leaky(x + bias) in ONE VectorE
  pass (and a _RED variant also emits the free-dim sum for pooling), so a
  DVE-owned tile costs one PSUM pass instead of three ops.
- Activations are fused across packs into [128, GROUP_COLS] groups (the
  conv biases are per-channel, identical for all packs), halving ScalarE
  per-op overhead. PSUM holds ZBUFS group-sized z tiles.
- Stage ownership (ScalarE Prelu vs VectorE custom op) is a tunable map;
  L1 and L5 default to DVE (L5's accum_out gives pooling for free, no
  ACTIVATION_READ_ACCUMULATOR), L2-4 mostly ScalarE.
- The head sigmoid is computed as 0.5*tanh(x/2)+0.5 so every ACT function
  used (identity/parametric_relu/tanh) lives in one table set - a single
  ACT_TABLE_LOAD, warmed by a dummy activation at t=0.
- All f32 constants arrive in ONE packed DMA (plus one f16 DMA for the
  conv weights) instead of 9 serialized transfers.
"""

import sys

if "/opt/trn_rl_repo" not in sys.path:
    sys.path.insert(0, "/opt/trn_rl_repo")

import numpy as np
from operator import add as _opadd

import concourse.mybir as mybir
from concourse.bacc import Bacc
from concourse import tile
from concourse.bass_utils import run_bass_kernel_spmd

B, N, H, W, C = 4, 128, 32, 32, 64
HW = H * W
NCORES = 8
PAIRS = (B * N) // NCORES  # 64 pairs per core
PACKS = PAIRS // 2  # 32 packed tiles per core
NEG = 0.1
F32 = mybir.dt.float32
F16 = mybir.dt.float16

CFG = dict(
    group_cols=2048,  # columns per fused activation group (1024 or 2048)
    zbufs=2,          # PSUM z-group buffers (zbufs * group_cols * 4B <= 16KB)
    hbufs=12,         # SBUF h-group buffers
    skew=2,           # wavefront skew between layers (in group waves)
    # (l, g) stages of layers 2-4 owned by VectorE instead of ScalarE
    dve_l24=6,
)

# ---------------------------------------------------------------------------
# Custom DVE ops: out = leaky(in0 + s0) with s1 = negative slope.
# Registered into concourse.dve_ops' registry at import (rows 17/18 of the
# 5-bit opcode field; 16 production ops occupy 1..16).
# ---------------------------------------------------------------------------
from concourse.dve_spec import Spec, Src0, C0, C1, maxx, lower, _has_src1, Zero
from concourse.dve_uop import DveOpSpec
from concourse import dve_ops as _dve_ops_mod
from concourse.dve_ops import DveOp


def _leaky_ref(in0, in1, s0, s1, imm2):
    y = in0.astype(np.float32) + s0
    return np.maximum(y, y * s1)


def _leaky_red_ref(in0, in1, s0, s1, imm2):
    b = _leaky_ref(in0, in1, s0, s1, imm2).astype(np.float32)
    return b, b.reshape(b.shape[0], -1).sum(axis=-1, keepdims=True)


def _register_leaky_ops():
    if "LEAKY_BIAS_ANT" in _dve_ops_mod._SUB_OPCODE_FOR_NAME:
        by_name = {op.name: op for op in _dve_ops_mod.OPS}
        return by_name["LEAKY_BIAS_ANT"], by_name["LEAKY_BIAS_RED_ANT"]
    body = maxx(Src0 + C0, (Src0 + C0) * C1)
    spec = Spec(body=body, reference=_leaky_ref)
    spec_r = Spec(body=body, accum=_opadd, accum_init=Zero, reference=_leaky_red_ref)
    ops = []
    for name, sp, row in (
        ("LEAKY_BIAS_ANT", spec, 17),
        ("LEAKY_BIAS_RED_ANT", spec_r, 18),
    ):
        shas = {}
        for ver in ("v3", "v4"):
            try:
                shas[ver] = DveOpSpec(
                    name=name, opcode=row, uops=lower(sp, ver=ver),
                    rd1_en=_has_src1(sp),
                ).sha(ver)
            except Exception:
                pass
        op = DveOp(name, sp, subdim=False, uops_sha=shas)
        _dve_ops_mod.OPS.append(op)
        _dve_ops_mod._SUB_OPCODE_FOR_NAME[name] = row
        _dve_ops_mod.CUSTOM_DVE_SPECS[name] = sp
        ops.append(op)
    return ops[0], ops[1]


_LEAKY_CACHE = _register_leaky_ops()
LEAKY, LEAKY_RED = _LEAKY_CACHE

A = mybir.ActivationFunctionType
OP = mybir.AluOpType

# packed f32 constants: column offsets within the [128, _PK_COLS] tile
_PK = {}
_off = 0
for _name, _p, _f in [
    ("xin", 5, HW),
    ("wu", 5, 128),
    ("wc", 4, 128),
    ("crd", 4, PACKS),
    ("bball", 128, 4),
    ("bb1", 128, 1),
    ("wh", 128, 6),
    ("bblh", 6, 1),
]:
    _PK[_name] = (_p, _off, _f)
    _off += _f
_PK_COLS = _off


def _dve_owned_l24():
    """Deterministic spread of layer-2..4 stages handed to VectorE."""
    ng = (PACKS * HW) // CFG["group_cols"]
    out = set()
    i = 0
    while len(out) < CFG["dve_l24"]:
        l = 2 + (i % 3)
        g = (i * 5 + 2) % ng
        out.add((l, g))
        i += 1
    return out


def _build():
    gc = CFG["group_cols"]
    ng = (PACKS * HW) // gc  # groups per layer
    ppg = gc // HW  # packs per group
    skew = CFG["skew"]
    dve_l24 = _dve_owned_l24()

    nc = Bacc()
    pk32_d = nc.dram_tensor("pk32", [128, _PK_COLS], F32, kind="ExternalInput")
    wall_d = nc.dram_tensor("wall", [128, 4 * 128], F16, kind="ExternalInput")
    out_d = nc.dram_tensor("out", [6, PACKS], F32, kind="ExternalOutput")

    with tile.TileContext(nc) as tc:
        with (
            tc.tile_pool(name="consts", bufs=1) as consts,
            tc.tile_pool(name="hpool", bufs=CFG["hbufs"]) as hpool,
            tc.tile_pool(name="s5pool", bufs=4) as s5pool,
            tc.tile_pool(name="zpool", bufs=CFG["zbufs"], space="PSUM") as zpool,
        ):
            # --- t=0: ACT table warm (dummy prelu+tanh on a memset tile) ---
            dw = consts.tile([128, 1], F32, tag="dw")
            nc.gpsimd.memset(dw[:], 0.0)
            dw2 = consts.tile([128, 1], F32, tag="dw2")
            nc.scalar.activation(dw2[:], dw[:], A.Prelu, scale=1.0, alpha=NEG)
            nc.scalar.activation(dw2[:], dw[:], A.Tanh)

            # --- const DMAs: one packed f32 + one f16 ---
            pk32 = consts.tile([128, _PK_COLS], F32, tag="pk32")
            nc.sync.dma_start(pk32[:], pk32_d[:])
            wall = consts.tile([128, 4 * 128], F16, tag="wall")
            nc.sync.dma_start(wall[:], wall_d[:])

            def pkap(name):
                p, off, f = _PK[name]
                return pk32[0:p, off : off + f]

            w_l = {l: wall[:, 128 * (l - 2) : 128 * (l - 1)] for l in (2, 3, 4, 5)}
            bb_l = {l: pkap("bball")[:, (l - 2) : (l - 1)] for l in (2, 3, 4, 5)}

            # --- per-pair layer-1 bias: bias1 = wc @ crd + b1  [128, PACKS] ---
            zpc = zpool.tile([128, PACKS], F32, tag="z", name="zpc")
            nc.tensor.matmul(zpc[:], pkap("wc"), pkap("crd"))
            bias1 = consts.tile([128, PACKS], F32, tag="bias1")
            nc.scalar.activation(bias1[:], zpc[:], A.Identity, bias=pkap("bb1"))

            # --- shared first-conv image term u [128, HW], duplicated halves,
            #     stored f16 so layer-1 DVE reads are cheap ---
            zpu = zpool.tile([128, HW], F32, tag="z", name="zpu")
            nc.tensor.matmul(zpu[:, 0:512], pkap("wu"), pkap("xin")[:, 0:512])
            nc.tensor.matmul(zpu[:, 512:1024], pkap("wu"), pkap("xin")[:, 512:1024])
            u_dup = consts.tile([128, HW], F16, tag="u_dup")
            nc.scalar.copy(u_dup[:, 0:512], zpu[:, 0:512])
            nc.vector.tensor_scalar(u_dup[:, 512:1024], zpu[:, 512:1024], 1.0, None, OP.mult)

            pooled = consts.tile([128, PACKS], F32, tag="pooled")

            hcur = {}

            def stage1(g):
                hg = hpool.tile([128, gc], F16, tag="h", name=f"h1_{g}")
                for i in range(ppg):
                    t = g * ppg + i
                    nc.vector._custom_dve(
                        LEAKY,
                        out=hg[:, HW * i : HW * (i + 1)],
                        in0=u_dup[:],
                        s0=bias1[:, t : t + 1],
                        s1=NEG,
                    )
                hcur[g] = hg

            def stage(l, g):
                hprev = hcur[g]
                z = zpool.tile([128, gc], F32, tag="z", name=f"z{l}_{g}")
                for c0 in range(0, gc, 512):
                    nc.tensor.matmul(
                        z[:, c0 : c0 + 512], w_l[l], hprev[:, c0 : c0 + 512],
                        start=True, stop=True, skip_group_check=True,
                    )
                if l == 5:
                    for i in range(ppg):
                        t = g * ppg + i
                        scr = s5pool.tile([128, HW], F16, tag="s5", name=f"s5_{t}")
                        nc.vector._custom_dve(
                            LEAKY_RED,
                            out=scr[:],
                            in0=z[:, HW * i : HW * (i + 1)],
                            s0=bb_l[5],
                            s1=NEG,
                            accum_out=pooled[:, t : t + 1],
                        )
                    return
                hg = hpool.tile([128, gc], F16, tag="h", name=f"h{l}_{g}")
                if (l, g) in dve_l24:
                    nc.vector._custom_dve(
                        LEAKY, out=hg[:], in0=z[:], s0=bb_l[l], s1=NEG
                    )
                else:
                    nc.scalar.activation(
                        hg[:], z[:], A.Prelu, bias=bb_l[l], scale=1.0, alpha=NEG
                    )
                hcur[g] = hg

            for w in range(ng + skew * 4):
                for l in (1, 2, 3, 4, 5):
                    g = w - skew * (l - 1)
                    if 0 <= g < ng:
                        stage1(g) if l == 1 else stage(l, g)

            # ---- head: sigmoid(x) = 0.5*tanh(0.5*x + 0.5*bl) + 0.5 ----
            zph = zpool.tile([6, PACKS], F32, tag="z", name="zph")
            nc.tensor.matmul(zph[:], pkap("wh"), pooled[:])
            th = consts.tile([6, PACKS], F32, tag="th")
            nc.scalar.activation(th[:], zph[:], A.Tanh, bias=pkap("bblh"), scale=0.5)
            out_sb = consts.tile([6, PACKS], F32, tag="out_sb")
            nc.vector.tensor_scalar(out_sb[:], th[:], 0.5, 0.5, OP.mult, OP.add)
            nc.sync.dma_start(out_d[:], out_sb[:])

    nc.compile()
    return nc


_CACHE = {}


def _get_nc():
    if "nc" not in _CACHE:
        _CACHE["nc"] = _build()
    return _CACHE["nc"]


def _prep_core_inputs(image, coords, w1, b1, ws, bs, wl, bl, core):
    b = core // 2
    n0 = (core % 2) * PAIRS

    row = (np.arange(H, dtype=np.float32) / (H - 1))[:, None] * np.ones(
        (1, W), np.float32
    )
    col = np.ones((H, 1), np.float32) * (np.arange(W, dtype=np.float32) / (W - 1))[None]
    pos = np.stack([row, col], 0).reshape(2, HW)
    xin = np.concatenate([image[b].reshape(3, HW), pos], 0)  # [5, HW]

    cs = coords[b, n0 : n0 + PAIRS]  # [64, 2]
    crd = np.stack([cs[0::2, 0], cs[0::2, 1], cs[1::2, 0], cs[1::2, 1]], 0)  # [4, 32]

    w1aT = np.ascontiguousarray(w1[:, :5].T)  # [5, 64]
    w1bT = np.ascontiguousarray(w1[:, 5:].T)  # [2, 64]
    wu = np.concatenate([w1aT, w1aT], 1)  # [5, 128]
    wc = np.zeros((4, 128), np.float32)
    wc[0:2, 0:64] = w1bT
    wc[2:4, 64:128] = w1bT

    wall = np.zeros((128, 4 * 128), np.float32)
    bball = np.zeros((128, 4), np.float32)
    for i, (w, bias) in enumerate(zip(ws, bs)):
        wall[0:64, 128 * i : 128 * i + 64] = w.T
        wall[64:128, 128 * i + 64 : 128 * i + 128] = w.T
        bball[:, i] = np.concatenate([bias, bias])

    wh = np.zeros((128, 6), np.float32)
    wh[0:64, 0:3] = wl.T / HW
    wh[64:128, 3:6] = wl.T / HW

    pk32 = np.zeros((128, _PK_COLS), np.float32)

    def put(name, arr):
        p, off, f = _PK[name]
        assert arr.shape == (p, f), (name, arr.shape)
        pk32[0:p, off : off + f] = arr

    put("xin", xin)
    put("wu", wu)
    put("wc", wc)
    put("crd", crd)
    put("bball", bball)
    put("bb1", np.concatenate([b1, b1]).reshape(128, 1))
    put("wh", wh)
    put("bblh", (np.concatenate([bl, bl]) / 2).reshape(6, 1))

    return {
        "pk32": np.ascontiguousarray(pk32, np.float32),
        "wall": wall.astype(np.float16),
    }


def _run(inputs, trace=False):
    image = np.asarray(inputs["image"], np.float32)
    coords = np.asarray(inputs["coords"], np.float32)
    w1 = np.asarray(inputs["w1"], np.float32)
    b1 = np.asarray(inputs["b1"], np.float32)
    ws = [np.asarray(inputs[f"w{i}"], np.float32) for i in (2, 3, 4, 5)]
    bs = [np.asarray(inputs[f"b{i}"], np.float32) for i in (2, 3, 4, 5)]
    wl = np.asarray(inputs["wl"], np.float32)
    bl = np.asarray(inputs["bl"], np.float32)

    nc = _get_nc()
    in_maps = [
        _prep_core_inputs(image, coords, w1, b1, ws, bs, wl, bl, c)
        for c in range(NCORES)
    ]
    res = run_bass_kernel_spmd(nc, in_maps, list(range(NCORES)), trace=trace)

    pred = np.empty((B, 3, N), np.float32)
    for c in range(NCORES):
        b = c // 2
        n0 = (c % 2) * PAIRS
        o = res.results[c]["out"]  # [6, 32]
        pred[b, :, n0 + 0 : n0 + PAIRS : 2] = o[0:3]
        pred[b, :, n0 + 1 : n0 + PAIRS : 2] = o[3:6]
    return pred, res


def kernel(**inputs) -> np.ndarray:
    pred, _ = _run(inputs, trace=False)
    return pred


# revision 5
# speedup vs baseline: 1.3115x; 1.3115x over previous
"""Trainium2 Bass kernel for nn_BilinearInterpolator (dense per-coord CNN).

Math (per (b, n) pair):
  u      = w1[:, :5] @ [image_b; pos]              # [64, 1024], shared over n
  v      = w1[:, 5:] @ coords[b, n] + b1           # [64] per-pair bias
  h1     = leaky(u + v)                            # [64, 1024]
  h_l    = leaky(W_l h_{l-1} + b_l)   l = 2..5
  pooled = mean_hw(h5);  out = sigmoid(wl @ pooled + bl)

Sharding: 512 (b, n) pairs data-parallel over 8 cores (64 pairs each; every
core owns a single b). On-chip layout packs 2 pairs per 128-partition tile
(channels 0-63 = even pair, 64-127 = odd pair); all matmuls use block-diagonal
[128, 128] weights.

v2 design:
- A custom DVE op (LEAKY_BIAS_ANT) computes leaky(x + bias) in ONE VectorE
  pass (and a _RED variant also emits the free-dim sum for pooling), so a
  DVE-owned tile costs one PSUM pass instead of three ops.
- Activations are fused across packs into [128, GROUP_COLS] groups (the
  conv biases are per-channel, identical for all packs), halving ScalarE
  per-op overhead. PSUM holds ZBUFS group-sized z tiles.
- Stage ownership (ScalarE Prelu vs VectorE custom op) is a tunable map;
  L1 and L5 default to DVE (L5's accum_out gives pooling for free, no
  ACTIVATION_READ_ACCUMULATOR), L2-4 mostly ScalarE.
- The head sigmoid is computed as 0.5*tanh(x/2)+0.5 so every ACT function
  used (identity/parametric_relu/tanh) lives in one table set - a single
  ACT_TABLE_LOAD, warmed by a dummy activation at t=0.
- All f32 constants arrive in ONE packed DMA (plus one f16 DMA for the
  conv weights) instead of 9 serialized transfers.
"""

import sys

if "/opt/trn_rl_repo" not in sys.path:
    sys.path.insert(0, "/opt/trn_rl_repo")

import numpy as np
from operator import add as _opadd

import concourse.mybir as mybir
from concourse.bacc import Bacc
from concourse import tile
from concourse.bass_utils import run_bass_kernel_spmd

B, N, H, W, C = 4, 128, 32, 32, 64
HW = H * W
NCORES = 8
PAIRS = (B * N) // NCORES  # 64 pairs per core
PACKS = PAIRS // 2  # 32 packed tiles per core
NEG = 0.1
F32 = mybir.dt.float32
F16 = mybir.dt.float16

CFG = dict(
    group_cols=1024,  # columns per fused activation group (1024 or 2048)
    zbufs=4,          # PSUM z-group buffers (zbufs * group_cols * 4B <= 16KB)
    hbufs=20,         # SBUF h-group buffers
    skew=3,           # wavefront skew between layers (in group waves)
    # (l, g) stages of layers 2-4 owned by VectorE instead of ScalarE
    dve_l24=16,
)

# ---------------------------------------------------------------------------
# Custom DVE ops: out = leaky(in0 + s0) with s1 = negative slope.
# Registered into concourse.dve_ops' registry at import (rows 17/18 of the
# 5-bit opcode field; 16 production ops occupy 1..16).
# ---------------------------------------------------------------------------
from concourse.dve_spec import Spec, Src0, C0, C1, maxx, lower, _has_src1, Zero
from concourse.dve_uop import DveOpSpec
from concourse import dve_ops as _dve_ops_mod
from concourse.dve_ops import DveOp


def _leaky_ref(in0, in1, s0, s1, imm2):
    y = in0.astype(np.float32) + s0
    return np.maximum(y, y * s1)


def _leaky_red_ref(in0, in1, s0, s1, imm2):
    b = _leaky_ref(in0, in1, s0, s1, imm2).astype(np.float32)
    return b, b.reshape(b.shape[0], -1).sum(axis=-1, keepdims=True)


def _register_leaky_ops():
    if "LEAKY_BIAS_ANT" in _dve_ops_mod._SUB_OPCODE_FOR_NAME:
        by_name = {op.name: op for op in _dve_ops_mod.OPS}
        return by_name["LEAKY_BIAS_ANT"], by_name["LEAKY_BIAS_RED_ANT"]
    body = maxx(Src0 + C0, (Src0 + C0) * C1)
    spec = Spec(body=body, reference=_leaky_ref)
    spec_r = Spec(body=body, accum=_opadd, accum_init=Zero, reference=_leaky_red_ref)
    ops = []
    for name, sp, row in (
        ("LEAKY_BIAS_ANT", spec, 17),
        ("LEAKY_BIAS_RED_ANT", spec_r, 18),
    ):
        shas = {}
        for ver in ("v3", "v4"):
            try:
                shas[ver] = DveOpSpec(
                    name=name, opcode=row, uops=lower(sp, ver=ver),
                    rd1_en=_has_src1(sp),
                ).sha(ver)
            except Exception:
                pass
        op = DveOp(name, sp, subdim=False, uops_sha=shas)
        _dve_ops_mod.OPS.append(op)
        _dve_ops_mod._SUB_OPCODE_FOR_NAME[name] = row
        _dve_ops_mod.CUSTOM_DVE_SPECS[name] = sp
        ops.append(op)
    return ops[0], ops[1]


_LEAKY_CACHE = _register_leaky_ops()
LEAKY, LEAKY_RED = _LEAKY_CACHE

A = mybir.ActivationFunctionType
OP = mybir.AluOpType

# packed f32 constants: column offsets within the [128, _PK_COLS] tile
_PK = {}
_off = 0
for _name, _p, _f in [
    ("xin", 5, HW),
    ("wu", 5, 128),
    ("wc", 4, 128),
    ("crd", 4, PACKS),
    ("bball", 128, 4),
    ("bb1", 128, 1),
    ("wh", 128, 6),
    ("bblh", 6, 1),
]:
    _PK[_name] = (_p, _off, _f)
    _off += _f
_PK_COLS = _off


def _dve_owned_l24():
    """Deterministic spread of layer-2..4 stages handed to VectorE."""
    ng = (PACKS * HW) // CFG["group_cols"]
    out = set()
    i = 0
    while len(out) < CFG["dve_l24"]:
        l = 2 + (i % 3)
        g = (i * 5 + 2) % ng
        out.add((l, g))
        i += 1
    return out


def _build():
    gc = CFG["group_cols"]
    ng = (PACKS * HW) // gc  # groups per layer
    ppg = gc // HW  # packs per group
    skew = CFG["skew"]
    dve_l24 = _dve_owned_l24()

    nc = Bacc()
    pk32_d = nc.dram_tensor("pk32", [128, _PK_COLS], F32, kind="ExternalInput")
    wall_d = nc.dram_tensor("wall", [128, 4 * 128], F16, kind="ExternalInput")
    out_d = nc.dram_tensor("out", [6, PACKS], F32, kind="ExternalOutput")

    with tile.TileContext(nc) as tc:
        with (
            tc.tile_pool(name="consts", bufs=1) as consts,
            tc.tile_pool(name="hpool", bufs=CFG["hbufs"]) as hpool,
            tc.tile_pool(name="s5pool", bufs=4) as s5pool,
            tc.tile_pool(name="zpool", bufs=CFG["zbufs"], space="PSUM") as zpool,
        ):
            # --- t=0: ACT table warm (dummy prelu+tanh on a memset tile) ---
            dw = consts.tile([128, 1], F32, tag="dw")
            nc.gpsimd.memset(dw[:], 0.0)
            dw2 = consts.tile([128, 1], F32, tag="dw2")
            nc.scalar.activation(dw2[:], dw[:], A.Prelu, scale=1.0, alpha=NEG)
            nc.scalar.activation(dw2[:], dw[:], A.Tanh)

            # --- const DMAs: one packed f32 + one f16 ---
            pk32 = consts.tile([128, _PK_COLS], F32, tag="pk32")
            nc.sync.dma_start(pk32[:], pk32_d[:])
            wall = consts.tile([128, 4 * 128], F16, tag="wall")
            nc.sync.dma_start(wall[:], wall_d[:])

            def pkap(name):
                p, off, f = _PK[name]
                return pk32[0:p, off : off + f]

            w_l = {l: wall[:, 128 * (l - 2) : 128 * (l - 1)] for l in (2, 3, 4, 5)}
            bb_l = {l: pkap("bball")[:, (l - 2) : (l - 1)] for l in (2, 3, 4, 5)}

            # --- per-pair layer-1 bias: bias1 = wc @ crd + b1  [128, PACKS] ---
            zpc = zpool.tile([128, PACKS], F32, tag="z", name="zpc")
            nc.tensor.matmul(zpc[:], pkap("wc"), pkap("crd"))
            bias1 = consts.tile([128, PACKS], F32, tag="bias1")
            nc.scalar.activation(bias1[:], zpc[:], A.Identity, bias=pkap("bb1"))

            # --- shared first-conv image term u [128, HW], duplicated halves,
            #     stored f16 so layer-1 DVE reads are cheap ---
            zpu = zpool.tile([128, HW], F32, tag="z", name="zpu")
            nc.tensor.matmul(zpu[:, 0:512], pkap("wu"), pkap("xin")[:, 0:512])
            nc.tensor.matmul(zpu[:, 512:1024], pkap("wu"), pkap("xin")[:, 512:1024])
            u_dup = consts.tile([128, HW], F16, tag="u_dup")
            nc.scalar.copy(u_dup[:, 0:512], zpu[:, 0:512])
            nc.vector.tensor_scalar(u_dup[:, 512:1024], zpu[:, 512:1024], 1.0, None, OP.mult)

            pooled = consts.tile([128, PACKS], F32, tag="pooled")

            hcur = {}

            def stage1(g):
                hg = hpool.tile([128, gc], F16, tag="h", name=f"h1_{g}")
                for i in range(ppg):
                    t = g * ppg + i
                    nc.vector._custom_dve(
                        LEAKY,
                        out=hg[:, HW * i : HW * (i + 1)],
                        in0=u_dup[:],
                        s0=bias1[:, t : t + 1],
                        s1=NEG,
                    )
                hcur[g] = hg

            def stage(l, g):
                hprev = hcur[g]
                z = zpool.tile([128, gc], F32, tag="z", name=f"z{l}_{g}")
                for c0 in range(0, gc, 512):
                    nc.tensor.matmul(
                        z[:, c0 : c0 + 512], w_l[l], hprev[:, c0 : c0 + 512],
                        start=True, stop=True, skip_group_check=True,
                    )
                if l == 5:
                    for i in range(ppg):
                        t = g * ppg + i
                        scr = s5pool.tile([128, HW], F16, tag="s5", name=f"s5_{t}")
                        nc.vector._custom_dve(
                            LEAKY_RED,
                            out=scr[:],
                            in0=z[:, HW * i : HW * (i + 1)],
                            s0=bb_l[5],
                            s1=NEG,
                            accum_out=pooled[:, t : t + 1],
                        )
                    return
                hg = hpool.tile([128, gc], F16, tag="h", name=f"h{l}_{g}")
                if (l, g) in dve_l24:
                    nc.vector._custom_dve(
                        LEAKY, out=hg[:], in0=z[:], s0=bb_l[l], s1=NEG
                    )
                else:
                    nc.scalar.activation(
                        hg[:], z[:], A.Prelu, bias=bb_l[l], scale=1.0, alpha=NEG
                    )
                hcur[g] = hg

            for w in range(ng + skew * 4):
                for l in (1, 2, 3, 4, 5):
                    g = w - skew * (l - 1)
                    if 0 <= g < ng:
                        stage1(g) if l == 1 else stage(l, g)

            # ---- head: sigmoid(x) = 0.5*tanh(0.5*x + 0.5*bl) + 0.5 ----
            zph = zpool.tile([6, PACKS], F32, tag="z", name="zph")
            nc.tensor.matmul(zph[:], pkap("wh"), pooled[:])
            th = consts.tile([6, PACKS], F32, tag="th")
            nc.scalar.activation(th[:], zph[:], A.Tanh, bias=pkap("bblh"), scale=0.5)
            out_sb = consts.tile([6, PACKS], F32, tag="out_sb")
            nc.vector.tensor_scalar(out_sb[:], th[:], 0.5, 0.5, OP.mult, OP.add)
            nc.sync.dma_start(out_d[:], out_sb[:])

    nc.compile()
    return nc


_CACHE = {}


def _get_nc():
    if "nc" not in _CACHE:
        _CACHE["nc"] = _build()
    return _CACHE["nc"]


def _prep_core_inputs(image, coords, w1, b1, ws, bs, wl, bl, core):
    b = core // 2
    n0 = (core % 2) * PAIRS

    row = (np.arange(H, dtype=np.float32) / (H - 1))[:, None] * np.ones(
        (1, W), np.float32
    )
    col = np.ones((H, 1), np.float32) * (np.arange(W, dtype=np.float32) / (W - 1))[None]
    pos = np.stack([row, col], 0).reshape(2, HW)
    xin = np.concatenate([image[b].reshape(3, HW), pos], 0)  # [5, HW]

    cs = coords[b, n0 : n0 + PAIRS]  # [64, 2]
    crd = np.stack([cs[0::2, 0], cs[0::2, 1], cs[1::2, 0], cs[1::2, 1]], 0)  # [4, 32]

    w1aT = np.ascontiguousarray(w1[:, :5].T)  # [5, 64]
    w1bT = np.ascontiguousarray(w1[:, 5:].T)  # [2, 64]
    wu = np.concatenate([w1aT, w1aT], 1)  # [5, 128]
    wc = np.zeros((4, 128), np.float32)
    wc[0:2, 0:64] = w1bT
    wc[2:4, 64:128] = w1bT

    wall = np.zeros((128, 4 * 128), np.float32)
    bball = np.zeros((128, 4), np.float32)
    for i, (w, bias) in enumerate(zip(ws, bs)):
        wall[0:64, 128 * i : 128 * i + 64] = w.T
        wall[64:128, 128 * i + 64 : 128 * i + 128] = w.T
        bball[:, i] = np.concatenate([bias, bias])

    wh = np.zeros((128, 6), np.float32)
    wh[0:64, 0:3] = wl.T / HW
    wh[64:128, 3:6] = wl.T / HW

    pk32 = np.zeros((128, _PK_COLS), np.float32)

    def put(name, arr):
        p, off, f = _PK[name]
        assert arr.shape == (p, f), (name, arr.shape)
        pk32[0:p, off : off + f] = arr

    put("xin", xin)
    put("wu", wu)
    put("wc", wc)
    put("crd", crd)
    put("bball", bball)
    put("bb1", np.concatenate([b1, b1]).reshape(128, 1))
    put("wh", wh)
    put("bblh", (np.concatenate([bl, bl]) / 2).reshape(6, 1))

    return {
        "pk32": np.ascontiguousarray(pk32, np.float32),
        "wall": wall.astype(np.float16),
    }


def _run(inputs, trace=False):
    image = np.asarray(inputs["image"], np.float32)
    coords = np.asarray(inputs["coords"], np.float32)
    w1 = np.asarray(inputs["w1"], np.float32)
    b1 = np.asarray(inputs["b1"], np.float32)
    ws = [np.asarray(inputs[f"w{i}"], np.float32) for i in (2, 3, 4, 5)]
    bs = [np.asarray(inputs[f"b{i}"], np.float32) for i in (2, 3, 4, 5)]
    wl = np.asarray(inputs["wl"], np.float32)
    bl = np.asarray(inputs["bl"], np.float32)

    nc = _get_nc()
    in_maps = [
        _prep_core_inputs(image, coords, w1, b1, ws, bs, wl, bl, c)
        for c in range(NCORES)
    ]
    res = run_bass_kernel_spmd(nc, in_maps, list(range(NCORES)), trace=trace)

    pred = np.empty((B, 3, N), np.float32)
    for c in range(NCORES):
        b = c // 2
        n0 = (c % 2) * PAIRS
        o = res.results[c]["out"]  # [6, 32]
        pred[b, :, n0 + 0 : n0 + PAIRS : 2] = o[0:3]
        pred[b, :, n0 + 1 : n0 + PAIRS : 2] = o[3:6]
    return pred, res


def kernel(**inputs) -> np.ndarray:
    pred, _ = _run(inputs, trace=False)
    return pred
